# revision 21
# baseline (speedup 1.0000x reference)
"""Trainium2 Bass kernel for a pre-LN transformer block (B=128, T=256, D=384, H=6).

Sharding: data-parallel over batch across 8 NeuronCores (16 batches/core).

Design notes:
- Matmuls run in bf16 (fp32 streams at 1/4 rate on the PE); residuals stay fp32.
- Activations are produced feature-major (hT) via PE transposes so every matmul
  contracts over the partition dim with K=128 chunks.
- LN rsqrt = exp(-0.5*ln(var+eps)); with softmax's exp this keeps every ACT
  function (ln/exp/relu/copy) inside the single natural_log_exp_and_others
  table set. get_activation_tables is pinned to that set so the table-load
  pass never flip-flops sets (each load costs ~1.3us).
- Causal mask is added into the score PSUM with an identity-weight matmul, so
  exp reads masked scores straight from PSUM; exp's accum_out yields row sums.
- Softmax normalization is deferred past attn@v: row-sum reciprocals are
  transposed to row layout (PE), broadcast via a DRAM bounce DMA, and applied
  during the oT PSUM->SBUF evacuation (one tensor_mul per head pair).
"""
import sys

for _p in ("/opt/trn_rl_repo",):
    if _p not in sys.path:
        sys.path.append(_p)

import numpy as np

import concourse.bacc as bacc
import concourse.bass as bass
import concourse.mybir as mybir
import concourse.tile as tile
from concourse.masks import make_causal_mask, make_identity

F32 = mybir.dt.float32
BF16 = mybir.dt.bfloat16
AF = mybir.ActivationFunctionType
ALU = mybir.AluOpType

N_CORES = 8
B, T, D, H, HD = 128, 256, 384, 6, 64
DF = 4 * D            # 1536
SB = B // N_CORES     # 16 batches per core
NEG = -1e9            # additive causal-mask value
EPS = 1e-5
PIN_SET = "natural_log_exp_and_others"

_orig_gat = bacc.get_activation_tables


def _pinned_gat(arch):
    tabs = _orig_gat(arch)
    fns = tabs.get(PIN_SET) or set()
    if AF.Exp in fns and AF.Ln in fns and AF.Relu in fns and AF.Copy in fns:
        tabs = {k: (v if k == PIN_SET else set()) for k, v in tabs.items()}
    return tabs


bacc.get_activation_tables = _pinned_gat


def build_program(reps: int = 1, use_bqkv=False, use_bp=False, use_b1=False, use_b2=False):
    nc = bacc.Bacc("TRN2", target_bir_lowering=False, debug=False)

    x_d = nc.dram_tensor("x", [SB, T, D], F32, kind="ExternalInput").ap()
    wqkv_d = nc.dram_tensor("wqkv", [3, 128, 3 * D], BF16, kind="ExternalInput").ap()
    wp_d = nc.dram_tensor("wp", [3, 128, D], BF16, kind="ExternalInput").ap()
    w1_d = nc.dram_tensor("w1", [3, 128, DF], BF16, kind="ExternalInput").ap()
    w2_d = nc.dram_tensor("w2", [12, 128, D], BF16, kind="ExternalInput").ap()
    bias_d = {}
    for name, use, n in (("bqkv", use_bqkv, 3 * D), ("bp", use_bp, D),
                         ("b1", use_b1, DF), ("b2", use_b2, D)):
        if use:
            bias_d[name] = nc.dram_tensor(name, [1, n], BF16, kind="ExternalInput").ap()
    rs_scr = nc.dram_tensor("rs_scr", [SB, 6, 256], F32).ap()  # internal scratch
    out_d = nc.dram_tensor("out", [SB, T, D], F32, kind="ExternalOutput").ap()

    with tile.TileContext(nc) as tc:
        _emit(nc, tc, x_d, wqkv_d, wp_d, w1_d, w2_d, bias_d, rs_scr, out_d, reps)
    nc.compile()
    return nc


def _emit(nc, tc, x_d, wqkv_d, wp_d, w1_d, w2_d, bias_d, rs_scr, out_d, reps):
    from contextlib import ExitStack
    ctx = ExitStack()
    with ctx:
        wpool = ctx.enter_context(tc.tile_pool(name="w", bufs=1))
        sb = ctx.enter_context(tc.tile_pool(name="sb", bufs=3))
        sbx = ctx.enter_context(tc.tile_pool(name="sbx", bufs=6))
        stats = ctx.enter_context(tc.tile_pool(name="stats", bufs=6))
        ps_mm = ctx.enter_context(tc.tile_pool(name="ps_mm", bufs=2, space="PSUM"))
        ps_sc = ctx.enter_context(tc.tile_pool(name="ps_sc", bufs=2, space="PSUM"))
        ps_tr = ctx.enter_context(tc.tile_pool(name="ps_tr", bufs=2, space="PSUM"))
        ps_ot = ctx.enter_context(tc.tile_pool(name="ps_ot", bufs=1, space="PSUM"))
        ps_rs = ctx.enter_context(tc.tile_pool(name="ps_rs", bufs=1, space="PSUM"))

        # --- constants ---
        for cval in (0.0, EPS):
            cap = wpool.tile([128, 1], F32, tag=f"const{cval}")
            nc.vector.memset(cap, cval)
            nc.const_aps.aps[(F32, cval)] = cap
        ident = wpool.tile([128, 128], BF16, tag="ident")
        make_identity(nc, ident)
        # transposed causal mask for S^T[ts, tq]: 0 where ts <= tq, NEG below diag
        trimaskT = wpool.tile([128, 128], BF16, tag="trimaskT")
        nc.gpsimd.memset(trimaskT, NEG)
        nc.gpsimd.affine_select(
            out=trimaskT, in_=trimaskT, compare_op=ALU.is_gt, fill=0.0,
            base=0, pattern=[[-1, 128]], channel_multiplier=1,
        )
        # per-head ones-selector columns for PSUM-row sums: sel6[:, h, j] = (j == h)
        sel6 = wpool.tile([128, 6, 6], BF16, tag="sel6")
        nc.gpsimd.memset(sel6, 0.0)
        for h in range(6):
            nc.gpsimd.memset(sel6[:, h, h : h + 1], 1.0)

        # --- weights ---
        wqkv_sb = wpool.tile([128, 3, 3 * D], BF16, tag="wqkv")
        wp_sb = wpool.tile([128, 3, D], BF16, tag="wp")
        w1_sb = wpool.tile([128, 3, DF], BF16, tag="w1")
        w2_sb = wpool.tile([128, 12, D], BF16, tag="w2")
        for c in range(3):
            nc.sync.dma_start(out=wqkv_sb[:, c, :], in_=wqkv_d[c])
            nc.sync.dma_start(out=wp_sb[:, c, :], in_=wp_d[c])
            nc.sync.dma_start(out=w1_sb[:, c, :], in_=w1_d[c])
        for c in range(12):
            nc.sync.dma_start(out=w2_sb[:, c, :], in_=w2_d[c])
        bias_sb = {}
        ones = None
        if bias_d:
            ones = wpool.tile([1, T], BF16, tag="ones")
            nc.vector.memset(ones, 1.0)
            for name, ap in bias_d.items():
                t = wpool.tile([1, ap.shape[1]], BF16, tag=f"b_{name}")
                nc.sync.dma_start(out=t, in_=ap)
                bias_sb[name] = t

        def ln_pre(x_tiles, key):
            """x_tiles: 2x [128, D] f32 -> normalized h tiles (bf16, token-major)."""
            from contextlib import ExitStack as _ES
            with tc.high_priority(offset=400):
                return _ln_pre_body(x_tiles, key)

        def _ln_pre_body(x_tiles, key):
            mv = stats.tile([128, 2, 2], F32, tag="mv")
            for tt in range(2):
                st = stats.tile([128, 6], F32, tag="st")
                nc.vector.bn_stats(out=st, in_=x_tiles[tt])
                nc.vector.bn_aggr(out=mv[:, tt, :], in_=st)
            lnv = stats.tile([128, 2], F32, tag="lnv")
            nc.scalar.activation(out=lnv, in_=mv[:, :, 1], func=AF.Ln, bias=EPS)
            rstd = stats.tile([128, 2], F32, tag="rstd")
            nc.scalar.activation(out=rstd, in_=lnv, func=AF.Exp, scale=-0.5)
            h_t = []
            for tt in range(2):
                h = sb.tile([128, D], BF16, tag=f"{key}{tt}")
                eng = nc.gpsimd if tt == 0 else nc.vector
                eng.tensor_scalar(
                    out=h, in0=x_tiles[tt],
                    scalar1=mv[:, tt, 0:1],
                    scalar2=rstd[:, tt : tt + 1],
                    op0=ALU.subtract, op1=ALU.mult,
                )
                h_t.append(h)
            return h_t

        def ln_tr(h_t, key, on_act=False):
            """h tiles -> hT [128, 3, T] bf16 (feature-major)."""
            hT = sb.tile([128, 3, T], BF16, tag=f"{key}T")
            for tt in range(2):
                trp = ps_tr.tile([128, 3, 128], BF16, tag="tr")
                for c in range(3):
                    nc.tensor.transpose(trp[:, c, :], h_t[tt][:, 128 * c : 128 * (c + 1)], ident)
                if on_act:
                    nc.scalar.copy(out=hT[:, :, 128 * tt : 128 * (tt + 1)], in_=trp)
                else:
                    nc.vector.tensor_copy(out=hT[:, :, 128 * tt : 128 * (tt + 1)], in_=trp)
            return hT

        x_tiles = {}
        st = {}

        def emit_x_dma(b):
            if b >= SB:
                return
            x_t = []
            for tt in range(2):
                xt = sbx.tile([128, D], F32, tag=f"x{tt}")
                nc.sync.dma_start(out=xt, in_=x_d[b, 128 * tt : 128 * (tt + 1), :])
                x_t.append(xt)
            x_tiles[b] = x_t

        def qkv(b):
            hT = ln_tr(st.pop(("h1", b)), "h", on_act=True)
            qkT = sb.tile([128, 6, T], BF16, tag="qkT")  # q: m 0..2, k: m 3..5
            nq = 4 if "bqkv" in bias_sb else 3
            for mp in range(3):  # pairs of m-chunks share one PSUM bank
                ps = ps_mm.tile([128, 512], F32, tag="mm")
                for sub in range(2):
                    m = 2 * mp + sub
                    reg = ps[:, 256 * sub : 256 * (sub + 1)]
                    for c in range(3):
                        nc.tensor.matmul(reg, lhsT=wqkv_sb[:, c, 128 * m : 128 * (m + 1)],
                                         rhs=hT[:, c, :], start=(c == 0), stop=(c == nq - 1))
                    if "bqkv" in bias_sb:
                        nc.tensor.matmul(reg, lhsT=bias_sb["bqkv"][:, 128 * m : 128 * (m + 1)],
                                         rhs=ones[:, :T], start=False, stop=True)
                nc.vector.tensor_copy(out=qkT[:, 2 * mp : 2 * mp + 2, :], in_=ps)
            v_sb = []
            for tt in range(2):
                ps = ps_mm.tile([128, D], F32, tag="mm")
                for c in range(3):
                    nc.tensor.matmul(ps, lhsT=hT[:, c, 128 * tt : 128 * (tt + 1)],
                                     rhs=wqkv_sb[:, c, 2 * D : 3 * D],
                                     start=(c == 0), stop=(c == nq - 1))
                if "bqkv" in bias_sb:
                    nc.tensor.matmul(ps, lhsT=ones[:, :128],
                                     rhs=bias_sb["bqkv"][:, 2 * D : 3 * D],
                                     start=False, stop=True)
                vt = sbx.tile([128, D], BF16, tag=f"v{tt}")
                nc.vector.tensor_copy(out=vt, in_=ps)
                v_sb.append(vt)
            st[("qkv", b)] = (qkT, v_sb)

        def attn_begin(b):
            qkT, v_sb = st.pop(("qkv", b))
            attnT = sb.tile([128, 6, 384], BF16, tag="attnT")  # [ts, blocks x tq]
            rs_ps = ps_rs.tile([6, 256], F32, tag="rs")
            st[("at", b)] = (qkT, v_sb, attnT, rs_ps, {})

        def attn_scores(b, h):
            """S^T layout per head: cols 0:256 = (ts c0) x (tq 0:256);
            cols 256:384 = (ts c1) x (tq c1)."""
            qkT, v_sb, attnT, rs_ps, S_t = st[("at", b)]
            m, po = h // 2, (h % 2) * 64
            qh = qkT[po : po + 64, m, :]
            kh = qkT[po : po + 64, 3 + m, :]
            S = ps_sc.tile([128, 384], F32, tag="sc")
            nc.tensor.matmul(S[:, 0:256], lhsT=kh[:, 0:128], rhs=qh,
                             start=True, stop=False)
            nc.tensor.matmul(S[:, 256:384], lhsT=kh[:, 128:256], rhs=qh[:, 128:256],
                             start=False, stop=False)
            nc.tensor.matmul(S[:, 0:128], lhsT=ident, rhs=trimaskT,
                             start=False, stop=False)
            nc.tensor.matmul(S[:, 256:384], lhsT=ident, rhs=trimaskT,
                             start=False, stop=True)
            S_t[h] = S

        def attn_finish(b, h):
            qkT, v_sb, attnT, rs_ps, S_t = st[("at", b)]
            S = S_t.pop(h)
            nc.scalar.activation(out=attnT[:, h, :], in_=S, func=AF.Exp)
            nc.tensor.matmul(rs_ps, lhsT=sel6[:, h, :], rhs=attnT[:, h, 0:256],
                             start=(h == 0), stop=False, skip_group_check=True)
            nc.tensor.matmul(rs_ps[:, 128:256], lhsT=sel6[:, h, :],
                             rhs=attnT[:, h, 256:384],
                             start=False, stop=(h == H - 1), skip_group_check=True)

        def attn_evac(b, h):
            pass

        def attn_rs(b):
            """rowsum reciprocals -> DRAM bounce broadcast to head-pair rows"""
            qkT, v_sb, attnT, rs_ps, S_t = st.pop(("at", b))
            rsi = stats.tile([6, 256], F32, tag="rsi")
            nc.vector.reciprocal(rsi, rs_ps)
            nc.sync.dma_start(out=rs_scr[b], in_=rsi)
            for m in range(3):
                src_ap = rs_scr[b, 2 * m : 2 * m + 2, :]
                src = bass.AP(tensor=src_ap.tensor, offset=src_ap.offset,
                              ap=[list(src_ap.ap[0]), [0, 64], list(src_ap.ap[1])])
                if m == 0:
                    bcs = []
                bc_m = sb.tile([128, 256], F32, tag=f"rsbc{m}")  # noqa
                nc.sync.dma_start(out=bc_m, in_=src)
                bcs.append(bc_m)
            st[("ov", b)] = (v_sb, attnT, bcs)

        def attn_out(b):
            """oT (normalized on evac) -> projection -> residual -> LN2 stats."""
            x_t = x_tiles.pop(b)
            v_sb, attnT, bcs = st.pop(("ov", b))
            oTs = sb.tile([128, 3, T], BF16, tag="oTs")
            for m in range(3):
                oT_ps = ps_ot.tile([128, T], F32, tag="ot")
                for sub in range(2):
                    h = 2 * m + sub
                    po = sub * 64
                    tp = (0, po)
                    nc.tensor.matmul(oT_ps[po : po + 64, :],
                                     lhsT=v_sb[0][:, HD * h : HD * (h + 1)],
                                     rhs=attnT[:, h, 0:256],
                                     start=True, stop=False, tile_position=tp)
                    nc.tensor.matmul(oT_ps[po : po + 64, 128:256],
                                     lhsT=v_sb[1][:, HD * h : HD * (h + 1)],
                                     rhs=attnT[:, h, 256:384],
                                     start=False, stop=True, tile_position=tp)
                nc.vector.tensor_mul(out=oTs[:, m, :], in0=oT_ps, in1=bcs[m])
            npj = 4 if "bp" in bias_sb else 3
            x2_t = []
            for tt in range(2):
                ps = ps_mm.tile([128, D], F32, tag="mm")
                for c in range(3):
                    nc.tensor.matmul(ps, lhsT=oTs[:, c, 128 * tt : 128 * (tt + 1)],
                                     rhs=wp_sb[:, c, :], start=(c == 0), stop=(c == npj - 1))
                if "bp" in bias_sb:
                    nc.tensor.matmul(ps, lhsT=ones[:, :128], rhs=bias_sb["bp"],
                                     start=False, stop=True)
                x2 = sb.tile([128, D], F32, tag=f"x2_{tt}")
                nc.vector.tensor_add(out=x2, in0=x_t[tt], in1=ps)
                x2_t.append(x2)
            st[("h2", b)] = ln_pre(x2_t, "g")
            st[("x2", b)] = x2_t

        def ffn_begin(b):
            h2T = ln_tr(st.pop(("h2", b)), "g")
            fT = sb.tile([128, 12, T], BF16, tag="fT")
            st[("f", b)] = (h2T, fT)

        def ffn_pair(b, mp, on_act=False):
            h2T, fT = st[("f", b)]
            nf = 4 if "b1" in bias_sb else 3
            ps = ps_mm.tile([128, 512], F32, tag="mm")
            for sub in range(2):
                m = 2 * mp + sub
                reg = ps[:, 256 * sub : 256 * (sub + 1)]
                for c in range(3):
                    nc.tensor.matmul(reg, lhsT=w1_sb[:, c, 128 * m : 128 * (m + 1)],
                                     rhs=h2T[:, c, :], start=(c == 0), stop=(c == nf - 1))
                if "b1" in bias_sb:
                    nc.tensor.matmul(reg, lhsT=bias_sb["b1"][:, 128 * m : 128 * (m + 1)],
                                     rhs=ones[:, :T], start=False, stop=True)
            if on_act:
                nc.scalar.activation(out=fT[:, 2 * mp : 2 * mp + 2, :], in_=ps, func=AF.Relu)
            else:
                nc.vector.tensor_scalar_max(out=fT[:, 2 * mp : 2 * mp + 2, :], in0=ps,
                                            scalar1=0.0)

        def ffn_end(b):
            h2T, fT = st.pop(("f", b))
            x2_t = st.pop(("x2", b))
            nf2 = 13 if "b2" in bias_sb else 12
            for tt in range(2):
                ps = ps_mm.tile([128, D], F32, tag="mm")
                for kc in range(12):
                    nc.tensor.matmul(ps, lhsT=fT[:, kc, 128 * tt : 128 * (tt + 1)],
                                     rhs=w2_sb[:, kc, :], start=(kc == 0), stop=(kc == nf2 - 1))
                if "b2" in bias_sb:
                    nc.tensor.matmul(ps, lhsT=ones[:, :128], rhs=bias_sb["b2"],
                                     start=False, stop=True)
                ot = sb.tile([128, D], F32, tag=f"o{tt}")
                nc.vector.tensor_add(out=ot, in0=x2_t[tt], in1=ps)
                nc.sync.dma_start(out=out_d[b, 128 * tt : 128 * (tt + 1), :], in_=ot)

        def merged(ra, tb):
            """Interleave attention heads of batch ra with FFN of batch tb."""
            if ra is not None:
                attn_begin(ra)
                for h in range(H):
                    attn_scores(ra, h)
                    if h == 1 and tb is not None:
                        ffn_begin(tb)
                    if h >= 1:
                        attn_finish(ra, h - 1)
                    if h >= 2 and tb is not None:
                        ffn_pair(tb, h - 2, on_act=True)
                attn_finish(ra, H - 1)
                attn_rs(ra)
            if tb is not None:
                if ra is None:
                    ffn_begin(tb)
                    for mp in range(6):
                        ffn_pair(tb, mp, on_act=True)
                else:
                    ffn_pair(tb, 4, on_act=True)
                    ffn_pair(tb, 5, on_act=True)
                ffn_end(tb)

        def emit_all():
            # stages per batch b: P=LN1 stats (slot b-1), Q=LN1 tr + QKV (slot b),
            # R=attention+rowsum bounce (slot b+1), S=oT/proj/resid/LN2 stats
            # (slot b+2), T=FFN (slot b+2, interleaved with R of b+1... offsets:
            # slot s runs Q(s), S(s-2), R(s-1) x T(s-2), P(s+1).
            emit_x_dma(0)
            emit_x_dma(1)
            st[("h1", 0)] = ln_pre(x_tiles[0], "h")
            for s in range(SB + 2):
                emit_x_dma(s + 2)
                if s + 1 < SB:
                    st[("h1", s + 1)] = ln_pre(x_tiles[s + 1], "h")
                if s < SB:
                    qkv(s)
                if s >= 2:
                    attn_out(s - 2)
                merged(s - 1 if 1 <= s <= SB else None,
                       s - 2 if s >= 2 else None)

        if reps == 1:
            emit_all()
        else:
            with tc.For_i(0, reps) as _:
                emit_all()


def prep_weights(Wq, Wk, Wv, Wp, bp, W1, b1, W2, b2, g1, be1, g2, be2):
    """Host-side weight folding. Returns dict of device arrays + bias flags."""
    import ml_dtypes
    bf = ml_dtypes.bfloat16
    Wq = np.asarray(Wq, np.float32)
    Wk = np.asarray(Wk, np.float32)
    Wv = np.asarray(Wv, np.float32)
    Wp = np.asarray(Wp, np.float32)
    W1 = np.asarray(W1, np.float32)
    W2 = np.asarray(W2, np.float32)
    g1 = np.asarray(g1, np.float32); be1 = np.asarray(be1, np.float32)
    g2 = np.asarray(g2, np.float32); be2 = np.asarray(be2, np.float32)
    bp = np.asarray(bp, np.float32); b1 = np.asarray(b1, np.float32)
    b2 = np.asarray(b2, np.float32)

    # [H, D, HD] -> [D, H*HD]
    Wq2 = Wq.transpose(1, 0, 2).reshape(D, D)
    Wk2 = Wk.transpose(1, 0, 2).reshape(D, D)
    Wv2 = Wv.transpose(1, 0, 2).reshape(D, D)
    Wqkv = np.concatenate([Wq2, Wk2, Wv2], axis=1)          # [D, 3D]
    bqkv = be1 @ Wqkv                                       # bias from LN1 beta
    Wqkv = g1[:, None] * Wqkv                               # fold LN1 gamma
    scale = 1.0 / np.sqrt(np.float32(D))
    Wqkv[:, :D] *= scale                                    # fold score scale into q
    bqkv = bqkv.copy()
    bqkv[:D] *= scale

    W1e = g2[:, None] * W1                                  # fold LN2 gamma
    b1e = b1 + be2 @ W1                                     # fold LN2 beta

    out = {
        "wqkv": np.ascontiguousarray(Wqkv.reshape(3, 128, 3 * D)).astype(bf),
        "wp": np.ascontiguousarray(Wp.reshape(3, 128, D)).astype(bf),
        "w1": np.ascontiguousarray(W1e.reshape(3, 128, DF)).astype(bf),
        "w2": np.ascontiguousarray(W2.reshape(12, 128, D)).astype(bf),
    }
    flags = {}
    for name, arr in (("bqkv", bqkv), ("bp", bp), ("b1", b1e), ("b2", b2)):
        if np.any(arr != 0):
            out[name] = arr.reshape(1, -1).astype(bf)
            flags[f"use_{name}"] = True
        else:
            flags[f"use_{name}"] = False
    return out, flags


_CACHE = {}


def get_program(flags, reps=1):
    key = (reps, tuple(sorted(flags.items())))
    if key not in _CACHE:
        _CACHE[key] = build_program(reps=reps, **flags)
    return _CACHE[key]


def make_in_maps(x, w):
    in_maps = []
    for c in range(N_CORES):
        m = {"x": np.ascontiguousarray(np.asarray(x, np.float32)[c * SB : (c + 1) * SB])}
        m.update(w)
        in_maps.append(m)
    return in_maps


def kernel(x, Wq, Wk, Wv, Wp, bp, W1, b1, W2, b2, g1, be1, g2, be2):
    from concourse.bass_utils import run_bass_kernel_spmd

    w, flags = prep_weights(Wq, Wk, Wv, Wp, bp, W1, b1, W2, b2, g1, be1, g2, be2)
    nc = get_program(flags, reps=1)
    in_maps = make_in_maps(x, w)
    res = run_bass_kernel_spmd(nc, in_maps, list(range(N_CORES)))
    return np.concatenate([res.results[c]["out"] for c in range(N_CORES)], axis=0)


# revision 22
# speedup vs baseline: 1.0914x; 1.0914x over previous
"""Trainium2 Bass kernel for a pre-LN transformer block (B=128, T=256, D=384, H=6).

Sharding: data-parallel over batch across 8 NeuronCores (16 batches/core).

Design notes:
- Matmuls run in bf16 (fp32 streams at 1/4 rate on the PE); residuals stay fp32.
- Activations are produced feature-major (hT) via PE transposes so every matmul
  contracts over the partition dim with K=128 chunks.
- LN rsqrt = exp(-0.5*ln(var+eps)); with softmax's exp this keeps every ACT
  function (ln/exp/relu/copy) inside the single natural_log_exp_and_others
  table set. get_activation_tables is pinned to that set so the table-load
  pass never flip-flops sets (each load costs ~1.3us).
- Causal mask is added into the score PSUM with an identity-weight matmul, so
  exp reads masked scores straight from PSUM; exp's accum_out yields row sums.
- Softmax normalization is deferred past attn@v: row-sum reciprocals are
  transposed to row layout (PE), broadcast via a DRAM bounce DMA, and applied
  during the oT PSUM->SBUF evacuation (one tensor_mul per head pair).
"""
import sys

for _p in ("/opt/trn_rl_repo",):
    if _p not in sys.path:
        sys.path.append(_p)

import numpy as np

import concourse.bacc as bacc
import concourse.bass as bass
import concourse.mybir as mybir
import concourse.tile as tile
from concourse.masks import make_causal_mask, make_identity

F32 = mybir.dt.float32
BF16 = mybir.dt.bfloat16
AF = mybir.ActivationFunctionType
ALU = mybir.AluOpType

N_CORES = 8
B, T, D, H, HD = 128, 256, 384, 6, 64
DF = 4 * D            # 1536
SB = B // N_CORES     # 16 batches per core
NEG = -1e9            # additive causal-mask value
EPS = 1e-5
PIN_SET = "natural_log_exp_and_others"

_orig_gat = bacc.get_activation_tables


def _pinned_gat(arch):
    tabs = _orig_gat(arch)
    fns = tabs.get(PIN_SET) or set()
    if AF.Exp in fns and AF.Ln in fns and AF.Relu in fns and AF.Copy in fns:
        tabs = {k: (v if k == PIN_SET else set()) for k, v in tabs.items()}
    return tabs


bacc.get_activation_tables = _pinned_gat


def build_program(reps: int = 1, use_bqkv=False, use_bp=False, use_b1=False, use_b2=False):
    nc = bacc.Bacc("TRN2", target_bir_lowering=False, debug=False)

    x_d = nc.dram_tensor("x", [SB, T, D], F32, kind="ExternalInput").ap()
    wqkv_d = nc.dram_tensor("wqkv", [3, 128, 3 * D], BF16, kind="ExternalInput").ap()
    wp_d = nc.dram_tensor("wp", [3, 128, D], BF16, kind="ExternalInput").ap()
    w1_d = nc.dram_tensor("w1", [3, 128, DF], BF16, kind="ExternalInput").ap()
    w2_d = nc.dram_tensor("w2", [12, 128, D], BF16, kind="ExternalInput").ap()
    bias_d = {}
    for name, use, n in (("bqkv", use_bqkv, 3 * D), ("bp", use_bp, D),
                         ("b1", use_b1, DF), ("b2", use_b2, D)):
        if use:
            bias_d[name] = nc.dram_tensor(name, [1, n], BF16, kind="ExternalInput").ap()
    rs_scr = nc.dram_tensor("rs_scr", [SB, 6, 256], F32).ap()  # internal scratch
    out_d = nc.dram_tensor("out", [SB, T, D], F32, kind="ExternalOutput").ap()

    with tile.TileContext(nc) as tc:
        _emit(nc, tc, x_d, wqkv_d, wp_d, w1_d, w2_d, bias_d, rs_scr, out_d, reps)
    nc.compile()
    return nc


def _emit(nc, tc, x_d, wqkv_d, wp_d, w1_d, w2_d, bias_d, rs_scr, out_d, reps):
    from contextlib import ExitStack
    ctx = ExitStack()
    with ctx:
        wpool = ctx.enter_context(tc.tile_pool(name="w", bufs=1))
        sb = ctx.enter_context(tc.tile_pool(name="sb", bufs=3))
        sbx = ctx.enter_context(tc.tile_pool(name="sbx", bufs=6))
        stats = ctx.enter_context(tc.tile_pool(name="stats", bufs=6))
        ps_mm = ctx.enter_context(tc.tile_pool(name="ps_mm", bufs=2, space="PSUM"))
        ps_sc = ctx.enter_context(tc.tile_pool(name="ps_sc", bufs=2, space="PSUM"))
        ps_tr = ctx.enter_context(tc.tile_pool(name="ps_tr", bufs=1, space="PSUM"))
        ps_ot = ctx.enter_context(tc.tile_pool(name="ps_ot", bufs=1, space="PSUM"))
        ps_rs = ctx.enter_context(tc.tile_pool(name="ps_rs", bufs=2, space="PSUM"))

        # --- constants ---
        for cval in (0.0, EPS):
            cap = wpool.tile([128, 1], F32, tag=f"const{cval}")
            nc.vector.memset(cap, cval)
            nc.const_aps.aps[(F32, cval)] = cap
        ident = wpool.tile([128, 128], BF16, tag="ident")
        make_identity(nc, ident)
        # transposed causal mask for S^T[ts, tq]: 0 where ts <= tq, NEG below diag
        trimaskT = wpool.tile([128, 128], BF16, tag="trimaskT")
        nc.gpsimd.memset(trimaskT, NEG)
        nc.gpsimd.affine_select(
            out=trimaskT, in_=trimaskT, compare_op=ALU.is_gt, fill=0.0,
            base=0, pattern=[[-1, 128]], channel_multiplier=1,
        )
        # per-head ones-selector columns for PSUM-row sums: sel6[:, h, j] = (j == h)
        sel6 = wpool.tile([128, 6, 6], BF16, tag="sel6")
        nc.gpsimd.memset(sel6, 0.0)
        for h in range(6):
            nc.gpsimd.memset(sel6[:, h, h : h + 1], 1.0)

        # --- weights ---
        wqkv_sb = wpool.tile([128, 3, 3 * D], BF16, tag="wqkv")
        wp_sb = wpool.tile([128, 3, D], BF16, tag="wp")
        w1_sb = wpool.tile([128, 3, DF], BF16, tag="w1")
        w2_sb = wpool.tile([128, 12, D], BF16, tag="w2")
        for c in range(3):
            nc.sync.dma_start(out=wqkv_sb[:, c, :], in_=wqkv_d[c])
            nc.sync.dma_start(out=wp_sb[:, c, :], in_=wp_d[c])
            nc.sync.dma_start(out=w1_sb[:, c, :], in_=w1_d[c])
        for c in range(12):
            nc.sync.dma_start(out=w2_sb[:, c, :], in_=w2_d[c])
        bias_sb = {}
        ones = None
        if bias_d:
            ones = wpool.tile([1, T], BF16, tag="ones")
            nc.vector.memset(ones, 1.0)
            for name, ap in bias_d.items():
                t = wpool.tile([1, ap.shape[1]], BF16, tag=f"b_{name}")
                nc.sync.dma_start(out=t, in_=ap)
                bias_sb[name] = t

        def ln_pre(x_tiles, key):
            """x_tiles: 2x [128, D] f32 -> normalized h tiles (bf16, token-major)."""
            from contextlib import ExitStack as _ES
            with tc.high_priority(offset=400):
                return _ln_pre_body(x_tiles, key)

        def _ln_pre_body(x_tiles, key):
            mv = stats.tile([128, 2, 2], F32, tag="mv")
            for tt in range(2):
                st = stats.tile([128, 6], F32, tag="st")
                nc.vector.bn_stats(out=st, in_=x_tiles[tt])
                nc.vector.bn_aggr(out=mv[:, tt, :], in_=st)
            lnv = stats.tile([128, 2], F32, tag="lnv")
            nc.scalar.activation(out=lnv, in_=mv[:, :, 1], func=AF.Ln, bias=EPS)
            rstd = stats.tile([128, 2], F32, tag="rstd")
            nc.scalar.activation(out=rstd, in_=lnv, func=AF.Exp, scale=-0.5)
            h_t = []
            for tt in range(2):
                h = sb.tile([128, D], BF16, tag=f"{key}{tt}")
                eng = nc.gpsimd if tt == 0 else nc.vector
                eng.tensor_scalar(
                    out=h, in0=x_tiles[tt],
                    scalar1=mv[:, tt, 0:1],
                    scalar2=rstd[:, tt : tt + 1],
                    op0=ALU.subtract, op1=ALU.mult,
                )
                h_t.append(h)
            return h_t

        def ln_tr(h_t, key, on_act=False):
            """h tiles -> hT [128, 3, T] bf16 (feature-major)."""
            hT = sb.tile([128, 3, T], BF16, tag=f"{key}T")
            for tt in range(2):
                trp = ps_tr.tile([128, 3, 128], BF16, tag="tr")
                for c in range(3):
                    nc.tensor.transpose(trp[:, c, :], h_t[tt][:, 128 * c : 128 * (c + 1)], ident)
                if on_act:
                    nc.scalar.copy(out=hT[:, :, 128 * tt : 128 * (tt + 1)], in_=trp)
                else:
                    nc.vector.tensor_copy(out=hT[:, :, 128 * tt : 128 * (tt + 1)], in_=trp)
            return hT

        x_tiles = {}
        st = {}

        def emit_x_dma(b):
            if b >= SB:
                return
            x_t = []
            for tt in range(2):
                xt = sbx.tile([128, D], F32, tag=f"x{tt}")
                nc.sync.dma_start(out=xt, in_=x_d[b, 128 * tt : 128 * (tt + 1), :])
                x_t.append(xt)
            x_tiles[b] = x_t

        def qkv(b):
            hT = ln_tr(st.pop(("h1", b)), "h", on_act=True)
            qkT = sb.tile([128, 6, T], BF16, tag="qkT")  # q: m 0..2, k: m 3..5
            nq = 4 if "bqkv" in bias_sb else 3
            for mp in range(3):  # pairs of m-chunks share one PSUM bank
                ps = ps_mm.tile([128, 512], F32, tag="mm")
                for sub in range(2):
                    m = 2 * mp + sub
                    reg = ps[:, 256 * sub : 256 * (sub + 1)]
                    for c in range(3):
                        nc.tensor.matmul(reg, lhsT=wqkv_sb[:, c, 128 * m : 128 * (m + 1)],
                                         rhs=hT[:, c, :], start=(c == 0), stop=(c == nq - 1))
                    if "bqkv" in bias_sb:
                        nc.tensor.matmul(reg, lhsT=bias_sb["bqkv"][:, 128 * m : 128 * (m + 1)],
                                         rhs=ones[:, :T], start=False, stop=True)
                nc.vector.tensor_copy(out=qkT[:, 2 * mp : 2 * mp + 2, :], in_=ps)
            v_sb = []
            for tt in range(2):
                ps = ps_mm.tile([128, D], F32, tag="mm")
                for c in range(3):
                    nc.tensor.matmul(ps, lhsT=hT[:, c, 128 * tt : 128 * (tt + 1)],
                                     rhs=wqkv_sb[:, c, 2 * D : 3 * D],
                                     start=(c == 0), stop=(c == nq - 1))
                if "bqkv" in bias_sb:
                    nc.tensor.matmul(ps, lhsT=ones[:, :128],
                                     rhs=bias_sb["bqkv"][:, 2 * D : 3 * D],
                                     start=False, stop=True)
                vt = sbx.tile([128, D], BF16, tag=f"v{tt}")
                nc.vector.tensor_copy(out=vt, in_=ps)
                v_sb.append(vt)
            st[("qkv", b)] = (qkT, v_sb)

        def attn_begin(b):
            qkT, v_sb = st.pop(("qkv", b))
            attnT = sb.tile([128, 6, 384], BF16, tag="attnT")  # [ts, blocks x tq]
            rs_ps = ps_rs.tile([6, 256], F32, tag="rs")
            st[("at", b)] = (qkT, v_sb, attnT, rs_ps, {})

        def attn_scores(b, h):
            """S^T layout per head: cols 0:256 = (ts c0) x (tq 0:256);
            cols 256:384 = (ts c1) x (tq c1)."""
            qkT, v_sb, attnT, rs_ps, S_t = st[("at", b)]
            m, po = h // 2, (h % 2) * 64
            qh = qkT[po : po + 64, m, :]
            kh = qkT[po : po + 64, 3 + m, :]
            S = ps_sc.tile([128, 384], F32, tag="sc")
            nc.tensor.matmul(S[:, 0:256], lhsT=kh[:, 0:128], rhs=qh,
                             start=True, stop=False)
            nc.tensor.matmul(S[:, 256:384], lhsT=kh[:, 128:256], rhs=qh[:, 128:256],
                             start=False, stop=False)
            nc.tensor.matmul(S[:, 0:128], lhsT=ident, rhs=trimaskT,
                             start=False, stop=False)
            nc.tensor.matmul(S[:, 256:384], lhsT=ident, rhs=trimaskT,
                             start=False, stop=True)
            S_t[h] = S

        def attn_finish(b, h):
            qkT, v_sb, attnT, rs_ps, S_t = st[("at", b)]
            S = S_t.pop(h)
            nc.scalar.activation(out=attnT[:, h, :], in_=S, func=AF.Exp)

        def attn_evac(b, h):
            pass

        def attn_rs(b):
            """rowsum reciprocals -> DRAM bounce broadcast to head-pair rows"""
            qkT, v_sb, attnT, rs_ps, S_t = st.pop(("at", b))
            for h in range(H):
                nc.tensor.matmul(rs_ps, lhsT=sel6[:, h, :], rhs=attnT[:, h, 0:256],
                                 start=(h == 0), stop=False, skip_group_check=True)
                nc.tensor.matmul(rs_ps[:, 128:256], lhsT=sel6[:, h, :],
                                 rhs=attnT[:, h, 256:384],
                                 start=False, stop=(h == H - 1), skip_group_check=True)
            rsi = stats.tile([6, 256], F32, tag="rsi")
            nc.vector.reciprocal(rsi, rs_ps)
            nc.sync.dma_start(out=rs_scr[b], in_=rsi)
            for m in range(3):
                src_ap = rs_scr[b, 2 * m : 2 * m + 2, :]
                src = bass.AP(tensor=src_ap.tensor, offset=src_ap.offset,
                              ap=[list(src_ap.ap[0]), [0, 64], list(src_ap.ap[1])])
                if m == 0:
                    bcs = []
                bc_m = sb.tile([128, 256], F32, tag=f"rsbc{m}")  # noqa
                nc.sync.dma_start(out=bc_m, in_=src)
                bcs.append(bc_m)
            st[("ov", b)] = (v_sb, attnT, bcs)

        def attn_out(b):
            """oT (normalized on evac) -> projection -> residual -> LN2 stats."""
            x_t = x_tiles.pop(b)
            v_sb, attnT, bcs = st.pop(("ov", b))
            oTs = sb.tile([128, 3, T], BF16, tag="oTs")
            for m in range(3):
                oT_ps = ps_ot.tile([128, T], F32, tag="ot")
                for sub in range(2):
                    h = 2 * m + sub
                    po = sub * 64
                    tp = (0, po)
                    nc.tensor.matmul(oT_ps[po : po + 64, :],
                                     lhsT=v_sb[0][:, HD * h : HD * (h + 1)],
                                     rhs=attnT[:, h, 0:256],
                                     start=True, stop=False, tile_position=tp)
                    nc.tensor.matmul(oT_ps[po : po + 64, 128:256],
                                     lhsT=v_sb[1][:, HD * h : HD * (h + 1)],
                                     rhs=attnT[:, h, 256:384],
                                     start=False, stop=True, tile_position=tp)
                nc.vector.tensor_mul(out=oTs[:, m, :], in0=oT_ps, in1=bcs[m])
            npj = 4 if "bp" in bias_sb else 3
            x2_t = []
            for tt in range(2):
                ps = ps_mm.tile([128, D], F32, tag="mm")
                for c in range(3):
                    nc.tensor.matmul(ps, lhsT=oTs[:, c, 128 * tt : 128 * (tt + 1)],
                                     rhs=wp_sb[:, c, :], start=(c == 0), stop=(c == npj - 1))
                if "bp" in bias_sb:
                    nc.tensor.matmul(ps, lhsT=ones[:, :128], rhs=bias_sb["bp"],
                                     start=False, stop=True)
                x2 = sb.tile([128, D], F32, tag=f"x2_{tt}")
                nc.vector.tensor_add(out=x2, in0=x_t[tt], in1=ps)
                x2_t.append(x2)
            st[("h2", b)] = ln_pre(x2_t, "g")
            st[("x2", b)] = x2_t

        def ffn_begin(b):
            h2T = ln_tr(st.pop(("h2", b)), "g")
            fT = sb.tile([128, 12, T], BF16, tag="fT")
            st[("f", b)] = (h2T, fT)

        def ffn_pair(b, mp, on_act=False):
            h2T, fT = st[("f", b)]
            nf = 4 if "b1" in bias_sb else 3
            ps = ps_mm.tile([128, 512], F32, tag="mm")
            for sub in range(2):
                m = 2 * mp + sub
                reg = ps[:, 256 * sub : 256 * (sub + 1)]
                for c in range(3):
                    nc.tensor.matmul(reg, lhsT=w1_sb[:, c, 128 * m : 128 * (m + 1)],
                                     rhs=h2T[:, c, :], start=(c == 0), stop=(c == nf - 1))
                if "b1" in bias_sb:
                    nc.tensor.matmul(reg, lhsT=bias_sb["b1"][:, 128 * m : 128 * (m + 1)],
                                     rhs=ones[:, :T], start=False, stop=True)
            if on_act:
                nc.scalar.activation(out=fT[:, 2 * mp : 2 * mp + 2, :], in_=ps, func=AF.Relu)
            else:
                nc.vector.tensor_scalar_max(out=fT[:, 2 * mp : 2 * mp + 2, :], in0=ps,
                                            scalar1=0.0)

        def ffn_end(b):
            h2T, fT = st.pop(("f", b))
            x2_t = st.pop(("x2", b))
            nf2 = 13 if "b2" in bias_sb else 12
            for tt in range(2):
                ps = ps_mm.tile([128, D], F32, tag="mm")
                for kc in range(12):
                    nc.tensor.matmul(ps, lhsT=fT[:, kc, 128 * tt : 128 * (tt + 1)],
                                     rhs=w2_sb[:, kc, :], start=(kc == 0), stop=(kc == nf2 - 1))
                if "b2" in bias_sb:
                    nc.tensor.matmul(ps, lhsT=ones[:, :128], rhs=bias_sb["b2"],
                                     start=False, stop=True)
                ot = sb.tile([128, D], F32, tag=f"o{tt}")
                nc.vector.tensor_add(out=ot, in0=x2_t[tt], in1=ps)
                nc.sync.dma_start(out=out_d[b, 128 * tt : 128 * (tt + 1), :], in_=ot)

        def merged(ra, tb):
            """Interleave attention heads of batch ra with FFN of batch tb."""
            if ra is not None:
                attn_begin(ra)
                for h in range(H):
                    attn_scores(ra, h)
                    if h == 1 and tb is not None:
                        ffn_begin(tb)
                    if h >= 1:
                        attn_finish(ra, h - 1)
                    if h >= 2 and tb is not None:
                        ffn_pair(tb, h - 2, on_act=True)
                attn_finish(ra, H - 1)
                attn_rs(ra)
            if tb is not None:
                if ra is None:
                    ffn_begin(tb)
                    for mp in range(6):
                        ffn_pair(tb, mp, on_act=True)
                else:
                    ffn_pair(tb, 4, on_act=True)
                    ffn_pair(tb, 5, on_act=True)
                ffn_end(tb)

        def emit_all():
            # stages per batch b: P=LN1 stats (slot b-1), Q=LN1 tr + QKV (slot b),
            # R=attention+rowsum bounce (slot b+1), S=oT/proj/resid/LN2 stats
            # (slot b+2), T=FFN (slot b+2, interleaved with R of b+1... offsets:
            # slot s runs Q(s), S(s-2), R(s-1) x T(s-2), P(s+1).
            emit_x_dma(0)
            emit_x_dma(1)
            st[("h1", 0)] = ln_pre(x_tiles[0], "h")
            for s in range(SB + 2):
                emit_x_dma(s + 2)
                if s + 1 < SB:
                    st[("h1", s + 1)] = ln_pre(x_tiles[s + 1], "h")
                if s < SB:
                    qkv(s)
                if s >= 2:
                    attn_out(s - 2)
                merged(s - 1 if 1 <= s <= SB else None,
                       s - 2 if s >= 2 else None)

        if reps == 1:
            emit_all()
        else:
            with tc.For_i(0, reps) as _:
                emit_all()


def prep_weights(Wq, Wk, Wv, Wp, bp, W1, b1, W2, b2, g1, be1, g2, be2):
    """Host-side weight folding. Returns dict of device arrays + bias flags."""
    import ml_dtypes
    bf = ml_dtypes.bfloat16
    Wq = np.asarray(Wq, np.float32)
    Wk = np.asarray(Wk, np.float32)
    Wv = np.asarray(Wv, np.float32)
    Wp = np.asarray(Wp, np.float32)
    W1 = np.asarray(W1, np.float32)
    W2 = np.asarray(W2, np.float32)
    g1 = np.asarray(g1, np.float32); be1 = np.asarray(be1, np.float32)
    g2 = np.asarray(g2, np.float32); be2 = np.asarray(be2, np.float32)
    bp = np.asarray(bp, np.float32); b1 = np.asarray(b1, np.float32)
    b2 = np.asarray(b2, np.float32)

    # [H, D, HD] -> [D, H*HD]
    Wq2 = Wq.transpose(1, 0, 2).reshape(D, D)
    Wk2 = Wk.transpose(1, 0, 2).reshape(D, D)
    Wv2 = Wv.transpose(1, 0, 2).reshape(D, D)
    Wqkv = np.concatenate([Wq2, Wk2, Wv2], axis=1)          # [D, 3D]
    bqkv = be1 @ Wqkv                                       # bias from LN1 beta
    Wqkv = g1[:, None] * Wqkv                               # fold LN1 gamma
    scale = 1.0 / np.sqrt(np.float32(D))
    Wqkv[:, :D] *= scale                                    # fold score scale into q
    bqkv = bqkv.copy()
    bqkv[:D] *= scale

    W1e = g2[:, None] * W1                                  # fold LN2 gamma
    b1e = b1 + be2 @ W1                                     # fold LN2 beta

    out = {
        "wqkv": np.ascontiguousarray(Wqkv.reshape(3, 128, 3 * D)).astype(bf),
        "wp": np.ascontiguousarray(Wp.reshape(3, 128, D)).astype(bf),
        "w1": np.ascontiguousarray(W1e.reshape(3, 128, DF)).astype(bf),
        "w2": np.ascontiguousarray(W2.reshape(12, 128, D)).astype(bf),
    }
    flags = {}
    for name, arr in (("bqkv", bqkv), ("bp", bp), ("b1", b1e), ("b2", b2)):
        if np.any(arr != 0):
            out[name] = arr.reshape(1, -1).astype(bf)
            flags[f"use_{name}"] = True
        else:
            flags[f"use_{name}"] = False
    return out, flags


_CACHE = {}


def get_program(flags, reps=1):
    key = (reps, tuple(sorted(flags.items())))
    if key not in _CACHE:
        _CACHE[key] = build_program(reps=reps, **flags)
    return _CACHE[key]


def make_in_maps(x, w):
    in_maps = []
    for c in range(N_CORES):
        m = {"x": np.ascontiguousarray(np.asarray(x, np.float32)[c * SB : (c + 1) * SB])}
        m.update(w)
        in_maps.append(m)
    return in_maps


def kernel(x, Wq, Wk, Wv, Wp, bp, W1, b1, W2, b2, g1, be1, g2, be2):
    from concourse.bass_utils import run_bass_kernel_spmd

    w, flags = prep_weights(Wq, Wk, Wv, Wp, bp, W1, b1, W2, b2, g1, be1, g2, be2)
    nc = get_program(flags, reps=1)
    in_maps = make_in_maps(x, w)
    res = run_bass_kernel_spmd(nc, in_maps, list(range(N_CORES)))
    return np.concatenate([res.results[c]["out"] for c in range(N_CORES)], axis=0)
